# revision 2
# baseline (speedup 1.0000x reference)
"""Causal self-attention (S=2048, B=4, D=768, H=12, Hd=64) on 8 TRN2 cores.

Sharding: core c -> (batch b = c//2, head-group hg = c%2).  Each core computes
the full attention for one batch element and 6 of the 12 heads, plus the Wo
projection restricted to its heads' columns; host sums the two head-group
partial outputs per batch.

Per-core kernel (all matmuls float32r = full PE rate at N>=256, ~1.6e-4 rel):
  xT [768, 2048] (host-pretransposed) -> SBUF as 6 tiles [128d, 2048s]
  qT/kT [384e, 2048s] = WqT/WkT-tile.T @ xT       (e on partition)
  v    [2048t, 6, 64e] = xT-tile.T @ WvT           (t on partition) + ones col
  per (head h, q-group g of 512):
    scoresT [128t, 512q] = kT-head-slice.T @ qT-head-slice   (K=64)
    (+ additive causal mask on diagonal blocks, in PSUM)
    P'T = exp(0.125 * scoresT)            (ACT, f32r out)
    out_aug [65, 512q] += v_aug-slice.T @ P'T   over t-blocks
      row 64 = softmax denominator (ones column trick)
    attnT-slice [64e, 512q] = out_aug[0:64] * broadcast(1/out_aug[64])
  y [2048s, 768] = attnT-tile.T @ WoT  (partial over this core's heads)
"""

import numpy as np

S = 2048
B = 4
D = 768
H_TOTAL = 12
HD = 64
H = 6          # heads per core
E = H * HD     # 384: local head-dim rows
ND = D // 128  # 6 d-tiles
NE = E // 128  # 3 e-tiles
NT = S // 128  # 16 t-tiles
NG = S // 512  # 4 q-groups
NEG = 1e30     # additive mask value (scale 0.125 applied inside exp)

_cached = None


def _build():
    import concourse.mybir as mybir
    import concourse.tile as tile
    from concourse import bacc

    f32 = mybir.dt.float32
    f32r = mybir.dt.float32r

    nc = bacc.Bacc("TRN2")

    xT_d = nc.dram_tensor("xT", [D, S], f32r, kind="ExternalInput")
    wq_d = nc.dram_tensor("WqT", [D, E], f32r, kind="ExternalInput")
    wk_d = nc.dram_tensor("WkT", [D, E], f32r, kind="ExternalInput")
    wv_d = nc.dram_tensor("WvT", [D, E], f32r, kind="ExternalInput")
    wo_d = nc.dram_tensor("WoT", [E, D], f32r, kind="ExternalInput")
    mask_d = nc.dram_tensor("mask", [128, 4, 512], f32, kind="ExternalInput")
    y_d = nc.dram_tensor("y", [S, D], f32, kind="ExternalOutput")

    with tile.TileContext(nc) as tc:
        with (
            tc.tile_pool(name="xt", bufs=6) as xt_pool,
            tc.tile_pool(name="w", bufs=1) as w_pool,
            tc.tile_pool(name="qk", bufs=6) as qk_pool,
            tc.tile_pool(name="vaug", bufs=16) as v_pool,
            tc.tile_pool(name="pt", bufs=4) as pt_pool,
            tc.tile_pool(name="ep", bufs=2) as ep_pool,
            tc.tile_pool(name="y", bufs=2) as y_pool,
            tc.tile_pool(name="psa", bufs=2, space="PSUM") as psa_pool,
            tc.tile_pool(name="pss", bufs=3, space="PSUM") as pss_pool,
            tc.tile_pool(name="pso", bufs=2, space="PSUM") as pso_pool,
        ):
            wq = w_pool.tile([128, ND, E], f32r, tag="wq")
            wk = w_pool.tile([128, ND, E], f32r, tag="wk")
            wv = w_pool.tile([128, ND, E], f32r, tag="wv")
            wo = w_pool.tile([128, NE, D], f32r, tag="wo")
            mask = w_pool.tile([128, 4, 512], f32, tag="mask")
            ones = w_pool.tile([128, 1], f32, tag="ones")

            nc.sync.dma_start(wq[:], wq_d.rearrange("(n p) e -> p n e", p=128))
            nc.sync.dma_start(wk[:], wk_d.rearrange("(n p) e -> p n e", p=128))
            nc.sync.dma_start(wv[:], wv_d.rearrange("(n p) e -> p n e", p=128))
            nc.sync.dma_start(wo[:], wo_d.rearrange("(n p) e -> p n e", p=128))
            nc.sync.dma_start(mask[:], mask_d[:])
            nc.vector.memset(ones[:], 1.0)

            xT = []
            for d in range(ND):
                t = xt_pool.tile([128, S], f32r, tag="xt")
                nc.sync.dma_start(t[:], xT_d[d * 128 : (d + 1) * 128, :])
                xT.append(t)

            # ---- v projection: v_aug[t] [128t, 6, 65] ----
            vaug = []
            for t in range(NT):
                ps = psa_pool.tile([128, E], f32, tag="psa")
                for d in range(ND):
                    nc.tensor.matmul(
                        ps[:],
                        xT[d][:, t * 128 : (t + 1) * 128],
                        wv[:, d, :],
                        start=(d == 0),
                        stop=(d == ND - 1),
                    )
                va = v_pool.tile([128, H, 65], f32r, tag="vaug")
                nc.vector.tensor_copy(
                    va[:, :, 0:64], ps[:].rearrange("p (h e) -> p h e", e=64)
                )
                nc.vector.tensor_copy(
                    va[:, :, 64:65], ones[:, None, :].to_broadcast((128, H, 1))
                )
                vaug.append(va)

            # ---- q/k projections per e-tile + attention per head-pair ----
            qT = [None] * NE
            kT = [None] * NE
            attnT = [None] * NE

            def project(dst_list, et, w_t, tag):
                tl = qk_pool.tile([128, S], f32r, tag="qk", name=f"{tag}T{et}")
                for ch in range(4):
                    ps = psa_pool.tile([128, 512], f32, tag="psa")
                    for d in range(ND):
                        nc.tensor.matmul(
                            ps[:],
                            w_t[:, d, et * 128 : (et + 1) * 128],
                            xT[d][:, ch * 512 : (ch + 1) * 512],
                            start=(d == 0),
                            stop=(d == ND - 1),
                        )
                    nc.vector.tensor_copy(tl[:, ch * 512 : (ch + 1) * 512], ps[:])
                dst_list[et] = tl

            def attend(h):
                et, po = h // 2, (h % 2) * 64
                for g in range(NG):
                    ntb = 4 * g + 4
                    po_out = pso_pool.tile([65, 512], f32, tag="pso")
                    pts = []
                    emitted_out = 0

                    def emit_scores(tb):
                        ps_s = pss_pool.tile([128, 512], f32, tag="pss")
                        nc.tensor.matmul(
                            ps_s[:],
                            kT[et][po : po + 64, tb * 128 : (tb + 1) * 128],
                            qT[et][po : po + 64, g * 512 : (g + 1) * 512],
                            start=True,
                            stop=True,
                        )
                        if tb >= 4 * g:
                            nc.vector.tensor_add(
                                ps_s[:], ps_s[:], mask[:, tb - 4 * g, :]
                            )
                        pt = pt_pool.tile([128, 512], f32r, tag="pt")
                        nc.scalar.activation(
                            pt[:],
                            ps_s[:],
                            mybir.ActivationFunctionType.Exp,
                            scale=0.125,
                        )
                        pts.append(pt)

                    def emit_out(tb):
                        nc.tensor.matmul(
                            po_out[:],
                            vaug[tb][:].rearrange("p h e -> p (h e)")[
                                :, h * 65 : (h + 1) * 65
                            ],
                            pts[tb][:],
                            start=(tb == 0),
                            stop=(tb == ntb - 1),
                        )

                    # software pipeline: keep one score/exp tile ahead of out
                    for tb in range(ntb):
                        emit_scores(tb)
                        if tb >= 1:
                            emit_out(emitted_out)
                            emitted_out += 1
                    while emitted_out < ntb:
                        emit_out(emitted_out)
                        emitted_out += 1

                    rcp = ep_pool.tile([1, 512], f32, tag="rcp")
                    nc.vector.reciprocal(rcp[:], po_out[64:65, :])
                    rb = ep_pool.tile([64, 512], f32, tag="rb")
                    nc.gpsimd.partition_broadcast(rb[:], rcp[:])
                    nc.vector.tensor_mul(
                        attnT[et][po : po + 64, g * 512 : (g + 1) * 512],
                        po_out[0:64, :],
                        rb[:],
                    )

            for et in range(NE):
                project(qT, et, wq, "q")
                project(kT, et, wk, "k")
                # attnT tiles share the xt pool slots (freed after projections)
                attnT[et] = xt_pool.tile([128, S], f32r, tag="xt", name=f"attnT{et}")
                attend(2 * et)
                attend(2 * et + 1)

            # ---- output projection ----
            for t in range(NT):
                ysb = y_pool.tile([128, D], f32, tag="y")
                for ch in range(2):
                    ps = psa_pool.tile([128, 384], f32, tag="psa")
                    for e in range(NE):
                        nc.tensor.matmul(
                            ps[:],
                            attnT[e][:, t * 128 : (t + 1) * 128],
                            wo[:, e, ch * 384 : (ch + 1) * 384],
                            start=(e == 0),
                            stop=(e == NE - 1),
                        )
                    nc.vector.tensor_copy(ysb[:, ch * 384 : (ch + 1) * 384], ps[:])
                nc.sync.dma_start(y_d[t * 128 : (t + 1) * 128, :], ysb[:])

    nc.compile()
    return nc


def _mask_np():
    # mask[tp, j, qf] = 0 if (tp <= qf - 128*j) else -NEG
    tp = np.arange(128)[:, None, None]
    j = np.arange(4)[None, :, None]
    qf = np.arange(512)[None, None, :]
    return np.where(tp <= qf - 128 * j, 0.0, -NEG).astype(np.float32)


def _in_maps(x, Wq, Wk, Wv, Wo):
    mask = _mask_np()
    maps = []
    for c in range(8):
        b, hg = c // 2, c % 2
        rows = slice(hg * E, (hg + 1) * E)
        maps.append(
            {
                "xT": np.ascontiguousarray(x[:, b, :].T),
                "WqT": np.ascontiguousarray(Wq[rows].T),
                "WkT": np.ascontiguousarray(Wk[rows].T),
                "WvT": np.ascontiguousarray(Wv[rows].T),
                "WoT": np.ascontiguousarray(Wo[:, rows].T),
                "mask": mask,
            }
        )
    return maps


def get_nc():
    global _cached
    if _cached is None:
        _cached = _build()
    return _cached


def kernel(x, Wq, Wk, Wv, Wo):
    from concourse.bass_utils import run_bass_kernel_spmd

    x = np.asarray(x, dtype=np.float32)
    nc = get_nc()
    res = run_bass_kernel_spmd(
        nc, _in_maps(x, Wq, Wk, Wv, Wo), core_ids=list(range(8))
    )
    out = np.empty((S, B, D), dtype=np.float32)
    for b in range(B):
        out[:, b, :] = res.results[2 * b]["y"] + res.results[2 * b + 1]["y"]
    return out


# revision 7
# speedup vs baseline: 1.0460x; 1.0460x over previous
"""Causal self-attention (S=2048, B=4, D=768, H=12, Hd=64) on 8 TRN2 cores.

Sharding: core c -> (batch b = c//2, head-group hg = c%2).  Each core computes
the full attention for one batch element and 6 of the 12 heads, plus the Wo
projection restricted to its heads' columns; host sums the two head-group
partial outputs per batch.

Per-core kernel (all matmuls float32r = full PE rate at N>=256, ~1.6e-4 rel):
  xT [768, 2048] (host-pretransposed) -> SBUF as 6 tiles [128d, 2048s]
  qT/kT [384e, 2048s] = WqT/WkT-tile.T @ xT       (e on partition)
  v    [2048t, 6, 64e] = xT-tile.T @ WvT           (t on partition) + ones col
  per (head h, q-group g of 512):
    scoresT [128t, 512q] = kT-head-slice.T @ qT-head-slice   (K=64)
    (+ additive causal mask on diagonal blocks, in PSUM)
    P'T = exp(0.125 * scoresT)            (ACT, f32r out)
    out_aug [65, 512q] += v_aug-slice.T @ P'T   over t-blocks
      row 64 = softmax denominator (ones column trick)
    attnT-slice [64e, 512q] = out_aug[0:64] * broadcast(1/out_aug[64])
  y [2048s, 768] = attnT-tile.T @ WoT  (partial over this core's heads)
"""

import numpy as np

S = 2048
B = 4
D = 768
H_TOTAL = 12
HD = 64
H = 6          # heads per core
E = H * HD     # 384: local head-dim rows
ND = D // 128  # 6 d-tiles
NE = E // 128  # 3 e-tiles
NT = S // 128  # 16 t-tiles
NG = S // 512  # 4 q-groups
NEG = 1e30     # additive mask value (scale 0.125 applied inside exp)

_cached = None


def _build():
    import concourse.mybir as mybir
    import concourse.tile as tile
    from concourse import bacc

    f32 = mybir.dt.float32
    f32r = mybir.dt.float32r

    nc = bacc.Bacc("TRN2")

    xT_d = nc.dram_tensor("xT", [D, S], f32r, kind="ExternalInput")
    wq_d = nc.dram_tensor("WqT", [D, E], f32r, kind="ExternalInput")
    wk_d = nc.dram_tensor("WkT", [D, E], f32r, kind="ExternalInput")
    wv_d = nc.dram_tensor("WvT", [D, E], f32r, kind="ExternalInput")
    wo_d = nc.dram_tensor("WoT", [E, D], f32r, kind="ExternalInput")
    mask_d = nc.dram_tensor("mask", [128, 4, 512], f32, kind="ExternalInput")
    y_d = nc.dram_tensor("y", [S, D], f32, kind="ExternalOutput")

    with tile.TileContext(nc) as tc:
        with (
            tc.tile_pool(name="xt", bufs=6) as xt_pool,
            tc.tile_pool(name="w", bufs=1) as w_pool,
            tc.tile_pool(name="qk", bufs=6) as qk_pool,
            tc.tile_pool(name="vaug", bufs=16) as v_pool,
            tc.tile_pool(name="pt", bufs=4) as pt_pool,
            tc.tile_pool(name="ep", bufs=3) as ep_pool,
            tc.tile_pool(name="y", bufs=2) as y_pool,
            tc.tile_pool(name="psa", bufs=2, space="PSUM") as psa_pool,
            tc.tile_pool(name="pss", bufs=3, space="PSUM") as pss_pool,
            tc.tile_pool(name="pso", bufs=3, space="PSUM") as pso_pool,
        ):
            wq = w_pool.tile([128, ND, E], f32r, tag="wq")
            wk = w_pool.tile([128, ND, E], f32r, tag="wk")
            wv = w_pool.tile([128, ND, E], f32r, tag="wv")
            wo = w_pool.tile([128, NE, D], f32r, tag="wo")
            mask = w_pool.tile([128, 4, 512], f32, tag="mask")
            dn0 = w_pool.tile([8, 512], f32, tag="dn0")
            dn1 = w_pool.tile([8, 512], f32, tag="dn1")
            dn2 = w_pool.tile([8, 512], f32, tag="dn2")
            dns = [dn0, dn1, dn2]
            ones = w_pool.tile([128, 1], f32, tag="ones")

            nc.sync.dma_start(wv[:], wv_d.rearrange("(n p) e -> p n e", p=128))
            xT = []
            for d in range(ND):
                t = xt_pool.tile([128, S], f32r, tag="xt")
                nc.sync.dma_start(t[:], xT_d[d * 128 : (d + 1) * 128, :])
                xT.append(t)
            nc.sync.dma_start(wq[:], wq_d.rearrange("(n p) e -> p n e", p=128))
            nc.sync.dma_start(wk[:], wk_d.rearrange("(n p) e -> p n e", p=128))
            nc.sync.dma_start(wo[:], wo_d.rearrange("(n p) e -> p n e", p=128))
            nc.sync.dma_start(mask[:], mask_d[:])
            nc.vector.memset(ones[:], 1.0)

            # ---- v projection: v_aug[t] [128t, 6, 65] ----
            vaug = []
            for t in range(NT):
                ps = psa_pool.tile([128, E], f32, tag="psa")
                for d in range(ND):
                    nc.tensor.matmul(
                        ps[:],
                        xT[d][:, t * 128 : (t + 1) * 128],
                        wv[:, d, :],
                        start=(d == 0),
                        stop=(d == ND - 1),
                    )
                va = v_pool.tile([128, H, 65], f32r, tag="vaug")
                nc.vector.tensor_copy(
                    va[:, :, 0:64], ps[:].rearrange("p (h e) -> p h e", e=64)
                )
                nc.vector.tensor_copy(
                    va[:, :, 64:65], ones[:, None, :].to_broadcast((128, H, 1))
                )
                vaug.append(va)

            # ---- q/k projections per e-tile + attention per head-pair ----
            qT = [None] * NE
            kT = [None] * NE
            attnT = [None] * NE

            def project(dst_list, et, w_t, tag):
                tl = qk_pool.tile([128, S], f32r, tag="qk", name=f"{tag}T{et}")
                for ch in range(4):
                    ps = psa_pool.tile([128, 512], f32, tag="psa")
                    for d in range(ND):
                        nc.tensor.matmul(
                            ps[:],
                            w_t[:, d, et * 128 : (et + 1) * 128],
                            xT[d][:, ch * 512 : (ch + 1) * 512],
                            start=(d == 0),
                            stop=(d == ND - 1),
                        )
                    nc.vector.tensor_copy(tl[:, ch * 512 : (ch + 1) * 512], ps[:])
                dst_list[et] = tl

            def attend(h):
                et, po = h // 2, (h % 2) * 64
                for g in range(NG):
                    ntb = 4 * g + 4
                    po_out = pso_pool.tile([65, 512], f32, tag="pso")
                    pts = []
                    emitted_out = 0

                    def emit_scores(tb):
                        ps_s = pss_pool.tile([128, 512], f32, tag="pss")
                        nc.tensor.matmul(
                            ps_s[:],
                            kT[et][po : po + 64, tb * 128 : (tb + 1) * 128],
                            qT[et][po : po + 64, g * 512 : (g + 1) * 512],
                            start=True,
                            stop=True,
                        )
                        if tb >= 4 * g:
                            j = tb - 4 * g
                            w = 128 * (j + 1)
                            nc.vector.tensor_add(
                                ps_s[:, 0:w], ps_s[:, 0:w], mask[:, j, 0:w]
                            )
                        pt = pt_pool.tile([128, 512], f32r, tag="pt")
                        nc.scalar.activation(
                            pt[:],
                            ps_s[:],
                            mybir.ActivationFunctionType.Exp,
                            scale=0.125,
                        )
                        pts.append(pt)

                    def emit_out(tb):
                        nc.tensor.matmul(
                            po_out[:],
                            vaug[tb][:].rearrange("p h e -> p (h e)")[
                                :, h * 65 : (h + 1) * 65
                            ],
                            pts[tb][:],
                            start=(tb == 0),
                            stop=(tb == ntb - 1),
                        )

                    # software pipeline: keep two score/exp tiles ahead of out
                    for tb in range(ntb):
                        emit_scores(tb)
                        if tb >= 2:
                            emit_out(emitted_out)
                            emitted_out += 1
                    while emitted_out < ntb:
                        emit_out(emitted_out)
                        emitted_out += 1

                    nc.vector.tensor_copy(
                        attnT[et][po : po + 64, g * 512 : (g + 1) * 512],
                        po_out[0:64, :],
                    )
                    idx = (h % 2) * NG + g
                    dtmp = ep_pool.tile([1, 512], f32, tag="dtmp")
                    nc.vector.tensor_copy(dtmp[:], po_out[64:65, :])
                    nc.sync.dma_start(dns[et][idx : idx + 1, :], dtmp[:])

            for et in range(NE):
                project(qT, et, wq, "q")
                project(kT, et, wk, "k")
                # attnT tiles share the xt pool slots (freed after projections)
                attnT[et] = xt_pool.tile([128, S], f32r, tag="xt", name=f"attnT{et}")
                attend(2 * et)
                attend(2 * et + 1)
                nc.vector.reciprocal(dns[et][0:8, :], dns[et][0:8, :])
                for h in (2 * et, 2 * et + 1):
                    po = (h % 2) * 64
                    for g in range(NG):
                        idx = (h % 2) * NG + g
                        tmp = ep_pool.tile([1, 512], f32, tag="tmp")
                        nc.sync.dma_start(tmp[:], dns[et][idx : idx + 1, :])
                        rb = ep_pool.tile([128, 512], f32, tag="rb")
                        nc.gpsimd.partition_broadcast(rb[:], tmp[:])
                        sl = attnT[et][po : po + 64, g * 512 : (g + 1) * 512]
                        nc.vector.tensor_mul(sl, sl, rb[po : po + 64, :])

            # ---- output projection ----
            for t in range(NT):
                ysb = y_pool.tile([128, D], f32, tag="y")
                for ch in range(2):
                    ps = psa_pool.tile([128, 384], f32, tag="psa")
                    for e in range(NE):
                        nc.tensor.matmul(
                            ps[:],
                            attnT[e][:, t * 128 : (t + 1) * 128],
                            wo[:, e, ch * 384 : (ch + 1) * 384],
                            start=(e == 0),
                            stop=(e == NE - 1),
                        )
                    nc.vector.tensor_copy(ysb[:, ch * 384 : (ch + 1) * 384], ps[:])
                nc.sync.dma_start(y_d[t * 128 : (t + 1) * 128, :], ysb[:])

    nc.compile()
    return nc


def _mask_np():
    # mask[tp, j, qf] = 0 if (tp <= qf - 128*j) else -NEG
    tp = np.arange(128)[:, None, None]
    j = np.arange(4)[None, :, None]
    qf = np.arange(512)[None, None, :]
    return np.where(tp <= qf - 128 * j, 0.0, -NEG).astype(np.float32)


def _in_maps(x, Wq, Wk, Wv, Wo):
    mask = _mask_np()
    maps = []
    for c in range(8):
        b, hg = c // 2, c % 2
        rows = slice(hg * E, (hg + 1) * E)
        maps.append(
            {
                "xT": np.ascontiguousarray(x[:, b, :].T),
                "WqT": np.ascontiguousarray(Wq[rows].T),
                "WkT": np.ascontiguousarray(Wk[rows].T),
                "WvT": np.ascontiguousarray(Wv[rows].T),
                "WoT": np.ascontiguousarray(Wo[:, rows].T),
                "mask": mask,
            }
        )
    return maps


def get_nc():
    global _cached
    if _cached is None:
        _cached = _build()
    return _cached


def kernel(x, Wq, Wk, Wv, Wo):
    from concourse.bass_utils import run_bass_kernel_spmd

    x = np.asarray(x, dtype=np.float32)
    nc = get_nc()
    res = run_bass_kernel_spmd(
        nc, _in_maps(x, Wq, Wk, Wv, Wo), core_ids=list(range(8))
    )
    out = np.empty((S, B, D), dtype=np.float32)
    for b in range(B):
        out[:, b, :] = res.results[2 * b]["y"] + res.results[2 * b + 1]["y"]
    return out
